# revision 93
# baseline (speedup 1.0000x reference)
"""Trainium2 Bass kernel for nn_Bottleneck_MDTA (B=16, C=256, H=W=64, heads=4).

Data-parallel over batch: 16 samples -> 8 cores x 2 samples, weights
replicated.  Channel-major layout [C on partitions, spatial free].

v3 (over the fp8-DoubleRow v2 baseline):
  * x is loaded from DRAM as bf16 straight into the residual buffer
    (halves input DMA and removes all residual-copy conversions); fp8
    xpad is converted from it.  Output is stored bf16 (halves out DMA,
    2x DVE mode on the proj evacuation).
  * The qkv-conv 1/aq scale is folded out of the PSUM evacuation: q,k
    are L2-normalized (scale-invariant) and v's scale is folded into
    the zrec row-prescale of the attention matrix, so all qkv conv
    evacuations are plain copies (594 ns on DVE instead of ~1.2 us).
  * tap9 of the depthwise conv is fused into the DVE evacuation for q,
    k AND v (PE runs 4 DoubleRow tap-pairs for all six blocks).
  * 1/sqrt for the q/k norms runs as exp(-0.5*ln(x)) so softmax Exp and
    the norm share one activation table (2 table loads per sample).
  * Preamble is parallel: split x DMA, pad-memsets and fp8 converts
    spread over DVE/Pool while weights stream in on the sync queue.
  * Two samples are software-pipelined: PE queue order is
    B0 C0 D0 cv1(1) G0 cv2(1) D1 G1 with activation buffers shared
    between samples (halves SBUF, WAR edges enforced by the framework).
  * All evacuations are distributed over ACT/DVE/Pool by static
    round-robin tables tuned against the timeline simulator.
"""

import numpy as np
import ml_dtypes

import concourse.bass as bass
import concourse.tile as tile
from concourse import bacc, mybir
from concourse.ap import AP
from concourse.bass_utils import run_bass_kernel_spmd
from concourse.hw_specs import get_activation_tables

BF = mybir.dt.bfloat16
F8 = mybir.dt.float8e4
F32 = mybir.dt.float32
AF = mybir.ActivationFunctionType
OP = mybir.AluOpType
AX = mybir.AxisListType
DR = mybir.MatmulPerfMode.DoubleRow

N_CORES = 8
S = 2            # samples per core
CB = 2           # channel blocks of 128 (C=256)
QKVB = 6         # qkv channel blocks (768)
P = 128
H = W = 64
HW = H * W
WS = 68          # padded row stride (W + 2 pad + 2 align)
C0 = 4           # interior column offset (left pad)
R0 = 1           # interior row offset (top pad)
NROWS = 67       # 66 real rows + 1 spare
PS = NROWS * WS  # padded buffer free size (4556)

# depthwise / cv2 tap pairs: k-subtile stride (delta) must be EVEN
TAP_PAIRS = [((0, 0), (0, 2)), ((1, 0), (1, 2)), ((2, 0), (2, 2)),
             ((0, 1), (1, 1))]
TAP9 = (2, 1)    # 9th tap: fused into the DVE stt evacuation

_CACHE = {}

# evacuation engine tables (indexable per-op; tuned against TimelineSim)
QKV_EVAC_TABLE = ["act", "dve"] * 12
AOP_EVAC_TABLE = ["act"] * 7 + ["dve"]
# per qkv-block dw evacuation: "stt" = 4 PE tap-pairs + tap9 fused into a
# DVE scalar_tensor_tensor; "act"/"dve" = 5 PE tap-pairs (tap9 zero-padded)
# + plain copy on that engine
DW_EVAC_TABLE = ["stt", "stt", "stt", "stt", "stt", "stt"]
# PE-queue interleaves: s0-era = cv2(s0) tail with D(s0) head; s1-era =
# G(s0) units ('g') with s1 cv1 ('1') / cv2 ('2') and D(s1) head ('D')
# (CoreSim's in-order interp flags the D-in-cv2 weave as reading y2p rows
# ahead of their writes, but the scheduled NEFF orders by dependency —
# hardware + TimelineSim both validate this schedule bit-identically to
# the un-woven one.)
WEAVE0_SCH = "22222D2D2D"
WEAVE_SCH = "1g1g1g1g" + "2g2g2g2g" + "2D2D2D2D"
# samples whose q-norm square is split into halves (shorter srow latency)
Q_SQUARE_SPLIT = ()
# engine pattern for the sample-0 xpad converts (indexed per (quarter, cb))
CONVERT0_ENGINES = ["dve", "pool"]
# stream the q/k norm squares per-hb inside the dw loop instead of one big
# square at qfin/kfin time (shortens the srow/invk critical chains)
STREAM_SQ = False
# which dw block's post-hook emits scores_b (4 = between v0/v1, 5 = after v1)
SCORES_B_AT = 4
# qkv blocks computed as a fused dense 3x3 (conv1x1*dw) straight from y2p:
# +PE matmuls, but removes the block's qkt round-trip evacuations entirely.
# Only v-blocks (4, 5) are eligible (q/k need the qkt staging for norms...
# actually any block works; v chosen since its PE cost lands at phase end).
# (fusing proved counterproductive: the v-block dw is on the PE critical
# path into the attention phase, so the added PE matmuls cost more than
# the removed evacuations; kept as an option, disabled.)
FUSED_BLOCKS = ()
FUSED_EVAC = ["act", "dve"]


class _Bacc(bacc.Bacc):
    """Bacc with the combined ln+exp activation table preferred, so the
    rsqrt-via-ln/exp norm and the softmax Exp share one table load."""

    def insert_act_table_loads(self):
        has_activation = any(
            isinstance(i, mybir.InstActivation)
            for b in self.main_func.blocks
            for i in b.instructions
        )
        if not has_activation:
            return
        # act_func_set_id is positional, so keep list order intact; blank
        # the ln-only set so Ln resolves to the combined ln+exp set instead.
        tables = []
        for name, funcs in get_activation_tables(self.m.arch).items():
            if name == "natural_log":
                funcs = type(funcs)()
            tables.append((name, funcs))
        import bass_rust as _bass_rust
        _bass_rust.insert_act_table_loads(self, tables)


def _off(pad_base, h0, dy, dx):
    """Padded-buffer offset for conv tap (dy,dx) reading out-rows h0..h0+7."""
    return pad_base + (h0 + dy) * WS + (C0 - 1) + dx


def build_bass():
    nc = _Bacc("TRN2", target_bir_lowering=False, debug=False,
               num_devices=N_CORES)

    # ---- DRAM I/O ----
    x_d = nc.dram_tensor("x", [S, 2 * P, HW], BF, kind="ExternalInput").ap()
    w1_d = nc.dram_tensor("w1", [P, 9, CB, P], F8, kind="ExternalInput").ap()
    w2_d = nc.dram_tensor("w2", [P, 5, CB, 2, P], F8, kind="ExternalInput").ap()
    wq_d = nc.dram_tensor("wq", [P, CB, QKVB, P], F8, kind="ExternalInput").ap()
    dw_d = nc.dram_tensor("dw", [P, QKVB, 5, 2, P], F8, kind="ExternalInput").ap()
    w9_d = nc.dram_tensor("w9", [P, QKVB], F32, kind="ExternalInput").ap()
    wp_d = nc.dram_tensor("wp", [P, CB, CB, P], F8, kind="ExternalInput").ap()
    w3_d = nc.dram_tensor("w3", [P, 2, 9, CB, P], F8,
                          kind="ExternalInput").ap()   # v-blocks only
    sc_d = nc.dram_tensor("sc", [P, 8], F32, kind="ExternalInput").ap()
    b2_d = nc.dram_tensor("b2", [P, CB], F32, kind="ExternalInput").ap()
    lnt_d = nc.dram_tensor("lnt", [P, CB], F32, kind="ExternalInput").ap()
    idn_d = nc.dram_tensor("idn", [P, P], BF, kind="ExternalInput").ap()
    out_d = nc.dram_tensor("out", [S, 2 * P, HW], BF,
                           kind="ExternalOutput").ap()

    # ---- persistent SBUF ----
    w1s = nc.alloc_sbuf_tensor("w1s", [P, 9, CB, P], F8).ap()
    w2s = nc.alloc_sbuf_tensor("w2s", [P, 5, CB, 2, P], F8).ap()
    wqs = nc.alloc_sbuf_tensor("wqs", [P, CB, QKVB, P], F8).ap()
    dws = nc.alloc_sbuf_tensor("dws", [P, QKVB, 5, 2, P], F8).ap()
    w9s = nc.alloc_sbuf_tensor("w9s", [P, QKVB], F32).ap()
    wps = nc.alloc_sbuf_tensor("wps", [P, CB, CB, P], F8).ap()
    w3s = nc.alloc_sbuf_tensor("w3s", [P, 2, 9, CB, P], F8).ap()
    scs = nc.alloc_sbuf_tensor("scs", [P, 8], F32).ap()
    b2s = nc.alloc_sbuf_tensor("b2s", [P, CB], F32).ap()
    lnts = nc.alloc_sbuf_tensor("lnts", [P, CB], F32).ap()
    idns = nc.alloc_sbuf_tensor("idns", [P, P], BF).ap()

    # activations (shared between the two pipelined samples)
    xres = nc.alloc_sbuf_tensor("xres", [P, S, CB, HW], BF).ap()
    xpad = nc.alloc_sbuf_tensor("xpad", [P, CB, PS], F8).ap()
    y1p = nc.alloc_sbuf_tensor("y1p", [P, PS], F8).ap()
    y2p = nc.alloc_sbuf_tensor("y2p", [P, CB, PS], F8).ap()
    qkt = nc.alloc_sbuf_tensor("qkt", [P, 2, PS], F8).ap()   # ping by qb%2

    kd = nc.alloc_sbuf_tensor("kd", [P, CB, HW], BF).ap()
    vd = nc.alloc_sbuf_tensor("vd", [P, CB, HW], BF).ap()
    qT = nc.alloc_sbuf_tensor("qT", [P, CB, HW], BF).ap()
    kT = nc.alloc_sbuf_tensor("kT", [P, CB, HW], BF).ap()
    attnE = nc.alloc_sbuf_tensor("attnE", [P, CB, P], BF).ap()
    attnTs = nc.alloc_sbuf_tensor("attnTs", [P, CB, P], BF).ap()
    sqs = nc.alloc_sbuf_tensor("sqs", [P, HW], BF).ap()      # square scratch

    qsq = nc.alloc_sbuf_tensor("qsq", [P, CB], F32).ap()
    ksq = nc.alloc_sbuf_tensor("ksq", [P, CB], F32).ap()
    qsqp = nc.alloc_sbuf_tensor("qsqp", [P, CB, 4], F32).ap()
    ksqp = nc.alloc_sbuf_tensor("ksqp", [P, CB, 4], F32).ap()
    lnq = nc.alloc_sbuf_tensor("lnq", [P, CB], F32).ap()
    lnk = nc.alloc_sbuf_tensor("lnk", [P, CB], F32).ap()
    srow = nc.alloc_sbuf_tensor("srow", [P, CB], F32).ap()
    invk = nc.alloc_sbuf_tensor("invk", [P, CB], F32).ap()
    zacc = nc.alloc_sbuf_tensor("zacc", [P, CB], F32).ap()
    zrec = nc.alloc_sbuf_tensor("zrec", [P, CB], F32).ap()

    b1v = scs[:, 0:1]
    s1v = scs[:, 1:2]
    s2v = scs[:, 2:3]
    pscv = scs[:, 3:4]
    aqv = scs[:, 4:5]

    XPITCH = CB * PS
    Y1PITCH = PS
    Y2PITCH = CB * PS
    QKPITCH = 2 * PS

    def pad_pair_rhs(tensor, pitch, base, h0, pa, pb):
        """DoubleRow rhs: 2 tap-shifted views of one padded buffer."""
        (dya, dxa), (dyb, dxb) = pa, pb
        oa = _off(base, h0, dya, dxa)
        d = (dyb - dya) * WS + (dxb - dxa)
        assert d % 2 == 0 and d != 0
        return AP(tensor.tensor, oa, [[pitch, P], [d, 2], [WS, 8], [1, 64]])

    def cb_pair_rhs(tensor, pitch, base, h0, dy, dx):
        """DoubleRow rhs: ksub = channel block (stride PS)."""
        return AP(tensor.tensor, _off(base, h0, dy, dx),
                  [[pitch, P], [PS, 2], [WS, 8], [1, 64]])

    def interior16(tensor, pitch, base, hb):
        """4-D [128, 2, 8, 64] interior view for out-rows hb*16..hb*16+15."""
        st = base + (hb * 16 + R0) * WS + C0
        return AP(tensor.tensor, st, [[pitch, P], [8 * WS, 2], [WS, 8], [1, 64]])

    def interior32(tensor, pitch, base, half):
        """3-D [128, 32, 64] interior view for rows half*32..half*32+31."""
        st = base + (half * 32 + R0) * WS + C0
        return AP(tensor.tensor, st, [[pitch, P], [WS, 32], [1, 64]])

    def shift21(tensor, pitch, base, hb):
        """Tap (2,1) input view matching interior16's 16 out-rows."""
        st = base + (hb * 16 + 2) * WS + (C0 - 1) + 1
        return AP(tensor.tensor, st, [[pitch, P], [8 * WS, 2], [WS, 8], [1, 64]])

    def flat4(tensor_ap):
        """[P, 1024] contiguous slice viewed as [P, 2, 8, 64]."""
        return tensor_ap.rearrange("p (a r c) -> p a r c", a=2, c=64)

    def cp4(cp):
        """[P, 2, 512] psum tile viewed as [P, 2, 8, 64]."""
        return cp.rearrange("p a (r c) -> p a r c", c=64)

    COPY = {"act": nc.scalar.copy,
            "dve": nc.vector.tensor_copy,
            "pool": nc.gpsimd.tensor_copy}
    MEMSET = {"dve": nc.vector.memset, "pool": nc.gpsimd.memset}

    # evacuation round-robin tables (per-op engine choice).
    # NOTE: Pool/GPSIMD cannot read PSUM, so PSUM evacuations may only go
    # to ACT or DVE.
    QKV_EVAC = QKV_EVAC_TABLE
    AOP_EVAC = AOP_EVAC_TABLE

    def _memset_pads(t2d, eng):
        MEMSET[eng](t2d[:, 0:WS], 0.0)
        MEMSET[eng](t2d[:, 65 * WS:67 * WS], 0.0)
        strip = t2d[:, WS:65 * WS].rearrange("p (r c) -> p r c", c=WS)[:, :, 0:C0]
        MEMSET[eng](strip, 0.0)

    with tile.TileContext(nc) as tc:
        with (
            tc.tile_pool(name="qe", bufs=2) as qep,            # q dw out, 4K
            tc.tile_pool(name="aop", bufs=3) as aopp,          # attn-out fp8
            tc.tile_pool(name="otp", bufs=2) as otpp,          # out bf16
            tc.tile_pool(name="cp", bufs=3, space="PSUM") as cpp,     # 2-bank
            tc.tile_pool(name="avp", bufs=2, space="PSUM") as avpp,   # 1-bank
        ):
            # ================= preamble =================
            # first two x quarters of each cb feed the first converts; w1
            # interleaves so cv1 can start ~4us in
            for q in range(2):
                for cb in range(CB):
                    nc.sync.dma_start(
                        xres[:, 0, cb, q * 1024:(q + 1) * 1024],
                        x_d[0, cb * P:(cb + 1) * P, q * 1024:(q + 1) * 1024])
            nc.sync.dma_start(w1s, w1_d)
            nc.sync.dma_start(scs, sc_d)
            nc.sync.dma_start(lnts, lnt_d)
            for q in range(2, 4):
                for cb in range(CB):
                    nc.sync.dma_start(
                        xres[:, 0, cb, q * 1024:(q + 1) * 1024],
                        x_d[0, cb * P:(cb + 1) * P, q * 1024:(q + 1) * 1024])
            # xpad pad memsets first (gate the s0 converts); the rest of the
            # pad memsets are emitted after the converts, they're needed
            # only once cv1/cv2/qkv run
            _memset_pads(xpad[:, 0, :], "dve")
            _memset_pads(xpad[:, 1, :], "pool")
            nc.gpsimd.memset(attnE[:, :, :], 0.0)
            # remaining weights + sample-1 x on the sync queue
            nc.sync.dma_start(w2s, w2_d)
            nc.sync.dma_start(wqs, wq_d)
            nc.sync.dma_start(dws, dw_d)
            nc.sync.dma_start(w9s, w9_d)
            nc.sync.dma_start(wps, wp_d)
            nc.sync.dma_start(w3s, w3_d)
            nc.sync.dma_start(b2s, b2_d)
            nc.sync.dma_start(idns, idn_d)
            for cb in range(CB):
                for half in range(2):
                    nc.sync.dma_start(
                        xres[:, 1, cb, half * 2048:(half + 1) * 2048],
                        x_d[1, cb * P:(cb + 1) * P,
                            half * 2048:(half + 1) * 2048])

            def emit_converts(s, engines):
                """xres bf16 -> xpad fp8, per (cb, quarter) [P, 1024]."""
                for i, (q, cb) in enumerate(
                        [(q, c) for q in range(4) for c in range(CB)]):
                    src = xres[:, s, cb, q * 1024:(q + 1) * 1024] \
                        .rearrange("p (r c) -> p r c", c=64)
                    dst = AP(xpad.tensor,
                             cb * PS + (q * 16 + R0) * WS + C0,
                             [[XPITCH, P], [WS, 16], [1, 64]])
                    COPY[engines[i % len(engines)]](dst, src)

            def gen_cv1(s):
                for hb in range(4):
                    cp = cpp.tile([P, 2, 512], F32, tag="ps")
                    for sub in range(2):
                        h0 = hb * 16 + sub * 8
                        for t in range(9):
                            dy, dx = t // 3, t % 3
                            nc.tensor.matmul(
                                cp[:, sub, :], w1s[:, t, :, :],
                                cb_pair_rhs(xpad, XPITCH, 0, h0, dy, dx),
                                start=(t == 0), stop=(t == 8), perf_mode=DR)
                    nc.scalar.activation(
                        interior16(y1p, Y1PITCH, 0, hb), cp4(cp),
                        AF.Silu, bias=b1v, scale=s1v)
                    yield

            def emit_cv1(s):
                for _ in gen_cv1(s):
                    pass

            def gen_cv2(s):
                # hb-major so stage-D convs can start after the first rows
                for hb in range(4):
                    for co in range(CB):
                        cp = cpp.tile([P, 2, 512], F32, tag="ps")
                        for sub in range(2):
                            h0 = hb * 16 + sub * 8
                            for pr in range(5):
                                if pr < 4:
                                    pa, pb = TAP_PAIRS[pr]
                                else:
                                    pa, pb = TAP9, (TAP9[0], TAP9[1] + 2)
                                nc.tensor.matmul(
                                    cp[:, sub, :], w2s[:, pr, co, :, :],
                                    pad_pair_rhs(y1p, Y1PITCH, 0, h0, pa, pb),
                                    start=(pr == 0), stop=(pr == 4),
                                    perf_mode=DR)
                        nc.scalar.activation(
                            interior16(y2p, Y2PITCH, co * PS, hb),
                            cp4(cp), AF.Silu,
                            bias=b2s[:, co:co + 1], scale=s2v)
                        yield

            def emit_cv2(s):
                for _ in gen_cv2(s):
                    pass

            # ===== stage D: qkv 1x1 conv + depthwise 3x3 =====
            def emit_conv(s, qb):
                ping = qb % 2
                for hb in range(4):
                    cp = cpp.tile([P, 2, 512], F32, tag="ps")
                    for sub in range(2):
                        h0 = hb * 16 + sub * 8
                        nc.tensor.matmul(
                            cp[:, sub, :], wqs[:, :, qb, :],
                            cb_pair_rhs(y2p, Y2PITCH, 0, h0, 1, 1),
                            start=True, stop=True, perf_mode=DR)
                    eng = QKV_EVAC[(s * 24 + qb * 4 + hb) % len(QKV_EVAC)]
                    dst = interior16(qkt, QKPITCH, ping * PS, hb)
                    if eng == "split":
                        # halves on ACT+DVE in parallel: the PSUM tile is
                        # released in ~660ns instead of ~1.2us
                        nc.scalar.copy(dst[:, 0:1, :, :], cp4(cp)[:, 0:1, :, :])
                        nc.vector.tensor_copy(dst[:, 1:2, :, :],
                                              cp4(cp)[:, 1:2, :, :])
                    else:
                        COPY[eng](dst, cp4(cp))

            def emit_dw(s, qb, qe_tile):
                ping = qb % 2
                kind, cb = qb // 2, qb % 2
                mode = DW_EVAC_TABLE[qb]
                npr = 4 if mode == "stt" else 5
                for hb in range(4):
                    cp = cpp.tile([P, 2, 512], F32, tag="ps")
                    for sub in range(2):
                        h0 = hb * 16 + sub * 8
                        for pr in range(npr):
                            if pr < 4:
                                pa, pb = TAP_PAIRS[pr]
                            else:
                                pa, pb = TAP9, (TAP9[0], TAP9[1] + 2)
                            nc.tensor.matmul(
                                cp[:, sub, :], dws[:, qb, pr, :, :],
                                pad_pair_rhs(qkt, QKPITCH, ping * PS,
                                             h0, pa, pb),
                                start=(pr == 0), stop=(pr == npr - 1),
                                perf_mode=DR)
                    if kind == 0:    # q -> qe pool tile
                        dst = qe_tile[:, hb * 1024:(hb + 1) * 1024]
                    elif kind == 1:  # k -> kd
                        dst = kd[:, cb, hb * 1024:(hb + 1) * 1024]
                    else:            # v -> vd
                        dst = vd[:, cb, hb * 1024:(hb + 1) * 1024]
                    if mode == "stt":
                        nc.vector.scalar_tensor_tensor(
                            flat4(dst), shift21(qkt, QKPITCH, ping * PS, hb),
                            w9s[:, qb:qb + 1], cp4(cp),
                            op0=OP.mult, op1=OP.add)
                    else:
                        COPY[mode](flat4(dst), cp4(cp))
                    if STREAM_SQ and kind < 2:
                        acc = (qsqp if kind == 0 else ksqp)[:, cb, hb:hb + 1]
                        nc.scalar.activation(
                            sqs[:, hb * 1024:(hb + 1) * 1024], dst,
                            AF.Square, accum_out=acc)

            def emit_dw_fused(s, qb):
                """v-block as dense 3x3 (conv1x1*dw folded host-side):
                reads y2p directly, writes vd; no qkt round-trip."""
                vb = qb - 4
                for hb in range(4):
                    cp = cpp.tile([P, 2, 512], F32, tag="ps")
                    for sub in range(2):
                        h0 = hb * 16 + sub * 8
                        for t in range(9):
                            dy, dx = t // 3, t % 3
                            nc.tensor.matmul(
                                cp[:, sub, :], w3s[:, vb, t, :, :],
                                cb_pair_rhs(y2p, Y2PITCH, 0, h0, dy, dx),
                                start=(t == 0), stop=(t == 8), perf_mode=DR)
                    COPY[FUSED_EVAC[(qb * 4 + hb) % len(FUSED_EVAC)]](
                        flat4(vd[:, qb % 2, hb * 1024:(hb + 1) * 1024]),
                        cp4(cp))

            def emit_kfin(s, cb):
                if STREAM_SQ:
                    nc.vector.tensor_reduce(
                        ksq[:, cb:cb + 1], ksqp[:, cb, :], axis=AX.X,
                        op=OP.add)
                else:
                    nc.scalar.activation(sqs, kd[:, cb, :], AF.Square,
                                         accum_out=ksq[:, cb:cb + 1])
                nc.vector.tensor_scalar_max(
                    ksq[:, cb:cb + 1], ksq[:, cb:cb + 1], 1e-24)
                nc.scalar.activation(lnk[:, cb:cb + 1], ksq[:, cb:cb + 1],
                                     AF.Ln)
                nc.scalar.activation(invk[:, cb:cb + 1], lnk[:, cb:cb + 1],
                                     AF.Exp, scale=-0.5)
                nc.vector.tensor_scalar_mul(
                    kd[:, cb, :], kd[:, cb, :], invk[:, cb:cb + 1])
                for j in range(2):
                    nc.sync.dma_start_transpose(
                        kT[:, cb, j * 2048:(j + 1) * 2048]
                        .rearrange("p (a b) -> p a b", b=P),
                        kd[:, cb, j * 2048:(j + 1) * 2048])

            def emit_qfin(s, cb, qe_tile):
                for j in range(2):
                    nc.sync.dma_start_transpose(
                        qT[:, cb, j * 2048:(j + 1) * 2048]
                        .rearrange("p (a b) -> p a b", b=P),
                        qe_tile[:, j * 2048:(j + 1) * 2048])
                if STREAM_SQ:
                    nc.vector.tensor_reduce(
                        qsq[:, cb:cb + 1], qsqp[:, cb, :], axis=AX.X,
                        op=OP.add)
                else:
                    nc.scalar.activation(sqs, qe_tile[:, :], AF.Square,
                                         accum_out=qsq[:, cb:cb + 1])
                nc.vector.tensor_scalar_max(
                    qsq[:, cb:cb + 1], qsq[:, cb:cb + 1], 1e-24)
                nc.scalar.activation(lnq[:, cb:cb + 1], qsq[:, cb:cb + 1],
                                     AF.Ln)
                nc.scalar.activation(srow[:, cb:cb + 1], lnq[:, cb:cb + 1],
                                     AF.Exp, scale=-0.5,
                                     bias=lnts[:, cb:cb + 1])

            def emit_scores_a(s):
                a0s = []
                for pb2 in range(CB):
                    a0 = avpp.tile([P, P], F32, name=f"a0{pb2}", tag="av")
                    for jj in range(HW // P):
                        nc.tensor.matmul(
                            a0, qT[:, pb2, jj * P:(jj + 1) * P],
                            kT[:, pb2, jj * P:(jj + 1) * P],
                            start=(jj == 0), stop=(jj == HW // P - 1))
                    a0s.append(a0)
                return a0s

            def emit_scores_b(s, a0s):
                for pb2 in range(CB):
                    for hh in range(2):
                        sl = slice(hh * 64, (hh + 1) * 64)
                        nc.scalar.activation(
                            attnE[sl, pb2, sl], a0s[pb2][sl, sl], AF.Exp,
                            scale=srow[sl, pb2:pb2 + 1],
                            accum_out=zacc[sl, pb2:pb2 + 1])
                    # v's aq scale is folded into wp/psc host-side, so
                    # zrec is a plain reciprocal
                    nc.vector.reciprocal(
                        zrec[:, pb2:pb2 + 1], zacc[:, pb2:pb2 + 1])
                    nc.vector.tensor_scalar_mul(
                        attnE[:, pb2, :], attnE[:, pb2, :],
                        zrec[:, pb2:pb2 + 1])
                    at = avpp.tile([P, P], BF, name="at", tag="av")
                    nc.tensor.transpose(at, attnE[:, pb2, :], idns)
                    nc.scalar.copy(attnTs[:, pb2, :], at)

            def gen_qkvdw(s):
                ORDER = [2, 3, 0, 1, 4, 5]
                qe_tiles = {}
                a0s_box = []

                def post(qb):
                    kind, cb = qb // 2, qb % 2
                    if kind == 1:
                        emit_kfin(s, cb)
                    elif kind == 0:
                        emit_qfin(s, cb, qe_tiles[cb])
                        if cb == 1:
                            a0s_box.append(emit_scores_a(s))
                    elif qb == SCORES_B_AT:
                        emit_scores_b(s, a0s_box[0])

                def do_dw(qb):
                    if qb in FUSED_BLOCKS:
                        emit_dw_fused(s, qb)
                    else:
                        emit_dw(s, qb, qe_tiles.get(qb % 2)
                                if qb // 2 == 0 else None)
                    post(qb)

                pending = None
                for qb in ORDER:
                    if qb // 2 == 0:
                        qe_tiles[qb % 2] = qep.tile([P, HW], BF, tag="qe",
                                                    name="qe")
                    if qb not in FUSED_BLOCKS:
                        emit_conv(s, qb)
                        yield
                    if pending is not None:
                        do_dw(pending)
                        yield
                    pending = qb
                do_dw(pending)

            def emit_qkvdw(s):
                for _ in gen_qkvdw(s):
                    pass

            def gen_attnproj(s):
                # one-stage software pipeline: attnv(j+1) is emitted before
                # proj(j) so PE never waits on the ACT ao-copy of chunk j
                def emit_av(j):
                    ao = aopp.tile([P, CB, 512], F8, name="ao")
                    av = cpp.tile([P, 2, 512], F32, tag="ps", name="av")
                    for pb in range(CB):
                        nc.tensor.matmul(
                            av[:, pb, :], attnTs[:, pb, :],
                            vd[:, pb, j * 512:(j + 1) * 512],
                            start=True, stop=True)
                    COPY[AOP_EVAC[j]](ao, av)
                    return ao

                def emit_proj(j, ao):
                    ot = otpp.tile([P, CB, 512], BF, name="ot")
                    for co in range(CB):
                        pp = avpp.tile([P, 512], F32, tag="av", name="pp")
                        nc.tensor.matmul(
                            pp, wps[:, :, co, :], ao[:, :, :],
                            start=True, stop=True, perf_mode=DR)
                        nc.vector.scalar_tensor_tensor(
                            ot[:, co, :], pp, pscv,
                            xres[:, s, co, j * 512:(j + 1) * 512],
                            op0=OP.mult, op1=OP.add)
                    nc.sync.dma_start(
                        out_d[s].rearrange("(a p) n -> p a n", a=CB)
                        [:, :, j * 512:(j + 1) * 512], ot)

                from collections import deque
                q = deque()
                for j in range(8):
                    q.append((j, emit_av(j)))
                    if len(q) > 2:
                        emit_proj(*q.popleft())
                    yield
                while q:
                    emit_proj(*q.popleft())

            def weave(gens, schedule):
                """Drive generators by schedule chars, then drain in order."""
                for ch in schedule:
                    next(gens[ch], None)
                for g in gens.values():
                    for _ in g:
                        pass

            # ================= pipeline =================
            emit_converts(0, CONVERT0_ENGINES)
            for i, pad2d in enumerate(
                    [y1p[:, :], y2p[:, 0, :], y2p[:, 1, :],
                     qkt[:, 0, :], qkt[:, 1, :]]):
                _memset_pads(pad2d, "dve" if i % 2 == 0 else "pool")
            emit_cv1(0)
            emit_converts(1, ["pool"])
            # overlap the tail of cv2(s0) with the head of stage D(s0)
            weave({"2": gen_cv2(0), "D": gen_qkvdw(0)}, WEAVE0_SCH)
            # overlap G(s0) with s1's cv1+cv2 on the PE queue so DVE/ACT
            # keep draining G's evacuations during s1's conv matmuls, and
            # the head of D(s1) with the tail of cv2(s1)
            weave({"1": gen_cv1(1), "g": gen_attnproj(0), "2": gen_cv2(1),
                   "D": gen_qkvdw(1)},
                  WEAVE_SCH)
            for _ in gen_attnproj(1):
                pass

    nc.compile()
    return nc


def _pow2scale(w, target=160.0):
    m = max(float(np.abs(w).max()), 1e-30)
    return float(2.0 ** np.floor(np.log2(target / m)))


def prep_inputs(inputs):
    """Host-side: fold BN, scale weights into e4m3 range, build lhsT tiles."""
    f = {k: np.asarray(v, dtype=np.float32) for k, v in inputs.items()}
    f8 = ml_dtypes.float8_e4m3

    s1 = f["cv1_g"] / np.sqrt(f["cv1_v"] + 1e-5)
    w1f = f["cv1_w"] * s1[:, None, None, None]          # [128, 256, 3, 3]
    b1 = (f["cv1_b"] - f["cv1_m"] * s1).astype(np.float32)
    a1 = _pow2scale(w1f)
    w1 = np.zeros((P, 9, CB, P), dtype=f8)
    for t in range(9):
        dy, dx = t // 3, t % 3
        for b in range(CB):
            w1[:, t, b, :] = (a1 * w1f[:, b * P:(b + 1) * P, dy, dx].T)

    s2 = f["cv2_g"] / np.sqrt(f["cv2_v"] + 1e-5)
    w2f = f["cv2_w"] * s2[:, None, None, None]          # [256, 128, 3, 3]
    b2v = f["cv2_b"] - f["cv2_m"] * s2
    a2 = _pow2scale(w2f)
    w2 = np.zeros((P, 5, CB, 2, P), dtype=f8)
    for pr in range(5):
        for co in range(CB):
            for k in range(2):
                if pr < 4:
                    dy, dx = TAP_PAIRS[pr][k]
                elif k == 0:
                    dy, dx = TAP9
                else:
                    continue
                w2[:, pr, co, k, :] = (
                    a2 * w2f[co * P:(co + 1) * P, :, dy, dx].T)
    b2 = np.ascontiguousarray(b2v.reshape(CB, P).T).astype(np.float32)

    # qkt holds the aq-scaled conv output directly (evac is a plain copy),
    # so aq is bounded by the fp8 range of the OUTPUT, not the weights.
    wqf = f["qkv_w"][:, :, 0, 0]                        # [768, 256]
    aq = min(_pow2scale(wqf), 32.0)
    wq = np.zeros((P, CB, QKVB, P), dtype=f8)
    for b in range(CB):
        for qb in range(QKVB):
            wq[:, b, qb, :] = (
                aq * wqf[qb * P:(qb + 1) * P, b * P:(b + 1) * P].T)

    dwf = f["dw_w"][:, 0, :, :]                         # [768, 3, 3]
    dwp = np.zeros((P, QKVB, 5, 2, P), dtype=f8)
    for qb in range(QKVB):
        for pr in range(5):
            for k in range(2):
                if pr < 4:
                    dy, dx = TAP_PAIRS[pr][k]
                elif k == 0:
                    dy, dx = TAP9
                else:
                    continue
                np.fill_diagonal(dwp[:, qb, pr, k, :],
                                 dwf[qb * P:(qb + 1) * P, dy, dx].astype(f8))
    w9 = np.zeros((P, QKVB), dtype=np.float32)
    for qb in range(QKVB):
        w9[:, qb] = dwf[qb * P:(qb + 1) * P, TAP9[0], TAP9[1]]

    # fused dense-3x3 weights for the v-blocks: W3[co,ci,t] = dw[co,t] *
    # (aq*wq[co,ci]); output scale matches the separate conv+dw path
    w3 = np.zeros((P, 2, 9, CB, P), dtype=f8)
    for vb in range(2):
        qb = 4 + vb
        rows = slice(qb * P, (qb + 1) * P)
        for t in range(9):
            dy, dx = t // 3, t % 3
            for b in range(CB):
                w3[:, vb, t, b, :] = (
                    aq * wqf[rows, b * P:(b + 1) * P]
                    * dwf[rows, dy, dx][:, None]).T

    # wp carries 1/aq so the attention-side aq scale (vd is aq-scaled)
    # cancels inside the proj matmul; psc = aq/ap_ compensates
    wpf = f["proj_w"][:, :, 0, 0]
    ap_ = _pow2scale(wpf)
    wp = np.zeros((P, CB, CB, P), dtype=f8)
    for b in range(CB):
        for ob in range(CB):
            wp[:, b, ob, :] = (
                (ap_ / aq) * wpf[ob * P:(ob + 1) * P, b * P:(b + 1) * P].T)

    temp = f["temperature"].reshape(4)
    lnt = np.zeros((P, CB), dtype=np.float32)
    for cb in range(CB):
        for p in range(P):
            lnt[p, cb] = np.log(max(temp[(cb * P + p) // 64], 1e-30))

    sc = np.zeros((P, 8), dtype=np.float32)
    sc[:, 0] = b1
    sc[:, 1] = 1.0 / a1
    sc[:, 2] = 1.0 / a2
    sc[:, 3] = 1.0 / ap_
    sc[:, 4] = aq

    return {
        "w1": w1, "w2": w2, "wq": wq, "dw": dwp, "w9": w9, "wp": wp,
        "w3": w3, "sc": sc, "b2": b2, "lnt": lnt,
        "idn": np.eye(P, dtype=ml_dtypes.bfloat16),
    }


def get_nc():
    if "nc" not in _CACHE:
        _CACHE["nc"] = build_bass()
    return _CACHE["nc"]


def kernel(**inputs):
    nc = get_nc()
    shared = prep_inputs(inputs)
    x = np.asarray(inputs["x"], dtype=np.float32)
    xb = x.reshape(16, 2 * P, HW).astype(ml_dtypes.bfloat16)
    in_maps = []
    for c in range(N_CORES):
        m = dict(shared)
        m["x"] = np.ascontiguousarray(xb[c * S:(c + 1) * S])
        in_maps.append(m)
    res = run_bass_kernel_spmd(nc, in_maps, core_ids=list(range(N_CORES)))
    out = np.concatenate([np.asarray(res.results[c]["out"])
                          .astype(np.float32) for c in range(N_CORES)],
                         axis=0)
    return out.reshape(16, 2 * P, H, W)
